# revision 23
# baseline (speedup 1.0000x reference)
"""Trainium2 Bass kernel for nn_Attention_23424751632639.

Computation (per (b,h)):  out = tril_strict(rope(Q) @ rope(Q).T / sqrt(N)) @ V
Chunked linear attention (exact reordering of the sums), chunk = 128 rows:
  out_c = QR_c @ M_{c-1}  +  strict_mask(QR_c @ QR_c^T) @ V_c
  M_c   = M_{c-1} + QR_c^T @ V_c          (M = running [64,64] state, PSUM)

Implementation (v3):
  * fp16 everywhere on device; all matmul accumulation stays fp32 in PSUM.
  * RoPE (elementwise) is applied on the host; the device receives QR in both
    natural [t, n] and transposed [n, t] layouts plus V, all fp16, pre-laid
    out per-partition so every DMA moves multi-KB contiguous runs (13 total
    dma_starts).  The scores scale N**-0.5 is folded into the rope tables.
  * Per chunk (4 heads) the PE runs: 4 state matmuls, 4 S blocks + 4 inter
    matmuls (S and inter share the same qrt stationary operand), 4 intra
    matmuls.  All matmul operands sit at partition base 0 (base-64 operands
    fault the device).
  * intra(c) is issued one chunk late so the strict-mask multiply (on
    DVE/ACT/GpSimd) never stalls the PE.
  * PSUM zero-region discipline: one start=True on the first write of each
    2KB region, one stop=True on the last; everything between accumulates.
  * PSUM->SBUF crossings (P-mask, M snapshot, output copy) are statically
    rotated across DVE / ACT / GpSimd.

Sharding: B*H = 32 (b,h) pairs -> 4 per core across 8 cores; no collectives.
"""

import math
import sys

import numpy as np

if "/opt/trn_rl_repo" not in sys.path:
    sys.path.insert(0, "/opt/trn_rl_repo")

B, H, T, N = 2, 16, 4096, 64
THETA = 2.0 ** 16
NCORES = 8
HPC = (B * H) // NCORES   # heads per core
CH = T // 128             # chunks per head (32)
NW = 4                    # windows
CPW = CH // NW            # chunks per window (8)
WCOLS = CPW * HPC * N     # columns per (window, stream) slice (2048)


def build_program():
    import concourse.mybir as mybir
    import concourse.tile as tile
    from concourse import bacc

    f32 = mybir.dt.float32
    f16 = mybir.dt.float16

    nc = bacc.Bacc(None, target_bir_lowering=False)
    # qn: [p, w, s, cw, h, n]; s: 0=qr 1=v       (natural layouts)
    qn = nc.dram_tensor("qn", [128, NW * 2 * WCOLS], f16, kind="ExternalInput")
    # qt: [p(n), w, cw, h, t] contiguous in DRAM; scattered into an SBUF
    # layout [w, cw, h, (slot 64 | t 128)] -- slot(c,h) holds the fp16 M
    # snapshot after chunk c-1 so [slot | t] is the merged S+inter rhs
    qt = nc.dram_tensor("qt", [64, NW * 2 * WCOLS], f16, kind="ExternalInput")
    cst = nc.dram_tensor("cst", [128, 512], f16, kind="ExternalInput")
    # o: [p, w, cw, h, n]
    o = nc.dram_tensor("o", [128, NW * WCOLS], f16, kind="ExternalOutput")

    with tile.TileContext(nc) as tc:
        with (
            tc.tile_pool(name="big", bufs=1) as bigp,
            tc.tile_pool(name="mb", bufs=2) as mbp,
            tc.tile_pool(name="psb", bufs=4) as psbp,
            tc.tile_pool(name="tmp", bufs=3) as tmpp,
            tc.tile_pool(name="ost", bufs=3) as ostp,
            tc.tile_pool(name="spps", bufs=3, space="PSUM") as spp,
            tc.tile_pool(name="outps", bufs=3, space="PSUM") as outp,
            tc.tile_pool(name="mps", bufs=1, space="PSUM") as mpp,
        ):
            qn_sb = bigp.tile([128, NW * 2 * WCOLS], f16)
            qt_sb = bigp.tile([64, NW * 3 * WCOLS], f16)
            cst_sb = bigp.tile([128, 512], f16)
            mask4 = cst_sb[:, 0:512]

            qn4 = qn.rearrange("p (w s q) -> p w s q", w=NW, s=2)
            qnsb4 = qn_sb.rearrange("p (w s q) -> p w s q", w=NW, s=2)
            qt5 = qt.rearrange("p (w c h t) -> p w c h t", w=NW, c=CPW, h=HPC)
            qtsb6 = qt_sb.rearrange("p (w c h b) -> p w c h b", w=NW, c=CPW, h=HPC)

            def dma_qn(w, clo, chi):
                a, b = 256 * clo, 256 * chi
                nc.sync.dma_start(qnsb4[:, w, :, a:b], qn4[:, w, :, a:b])

            def dma_qt(w, clo, chi):
                nc.sync.dma_start(qtsb6[:, w, clo:chi, :, 64:192],
                                  qt5[:, w, clo:chi, :, :])

            mreg = mpp.tile([64, 256], f32, name="mreg")

            # sp head-block offsets: h0,h1 in bank0; h2,h3 in bank1 (padded)
            HOFF = [0, 192, 512, 704]

            # per-chunk records for the 2-chunk-lagged intra
            rec = {}
            ost_t = [None] * NW

            def body(c):
                w, cl = c // CPW, c % CPW
                base = w * 2 * WCOLS
                qtb = 3 * w * WCOLS + 768 * cl  # window/chunk base in qt_sb

                def qr_sl(h):  # [128, 64] natural rope(Q) chunk
                    off = base + 256 * cl + 64 * h
                    return qn_sb[:, off:off + 64]

                def v_sl(h):   # [128, 64] V chunk
                    off = base + WCOLS + 256 * cl + 64 * h
                    return qn_sb[:, off:off + 64]

                # state: M_h += QR_c^T V_c   (PSUM accumulate across chunks)
                for h in range(HPC):
                    nc.tensor.matmul(
                        mreg[:, 64 * h:64 * h + 64],
                        qr_sl(h), v_sl(h),
                        start=(c == 0 and h == 0),
                        stop=(c == CH - 1 and h == HPC - 1),
                        skip_group_check=True,
                    )

                # M snapshot into chunk c+1's qt slots (rhs prefix for the
                # merged S+inter matmul of chunk c+1)
                if c < CH - 1:
                    w1, cl1 = (c + 1) // CPW, (c + 1) % CPW
                    dst = qtsb6[:, w1, cl1, :, 0:64]
                    if c % 2 == 0:
                        nc.vector.tensor_copy(dst, mreg[:])
                    else:
                        nc.scalar.copy(dst, mreg[:])

                # merged S+inter: out[:, 0:64] = qrt^T @ mb, [:, 64:192] = S
                sp = spp.tile([128, 1024], f32, tag="sp")
                for h in range(HPC):
                    lhs = qt_sb[:, qtb + 192 * h + 64:qtb + 192 * h + 192]
                    if c > 0:
                        nc.tensor.matmul(
                            sp[:, HOFF[h]:HOFF[h] + 192],
                            lhs, qt_sb[:, qtb + 192 * h:qtb + 192 * h + 192],
                            start=(h % 2 == 0), stop=False,
                            skip_group_check=True,
                        )
                    else:
                        nc.tensor.matmul(
                            sp[:, HOFF[h] + 64:HOFF[h] + 192],
                            lhs, lhs,
                            start=(h % 2 == 0), stop=False,
                            skip_group_check=True,
                        )

                # P = S * strict-upper mask  (psum f32 -> sbuf fp16)
                sps = sp.rearrange("p (g q) -> p g q", g=2)[:, :, 0:384]
                sps = sps.rearrange("p g (j b) -> p g j b", j=2)[:, :, :, 64:192]
                psb = psbp.tile([128, 512], f16, tag="psb")
                psb4 = psb.rearrange("p (g j n) -> p g j n", g=2, j=2)
                mk4 = mask4.rearrange("p (g j n) -> p g j n", g=2, j=2)
                r = c % 4
                if r in (1, 3):
                    nc.vector.tensor_mul(psb4, sps, mk4)
                else:
                    tmp = tmpp.tile([128, 512], f16, tag="tmp")
                    tmp4 = tmp.rearrange("p (g j n) -> p g j n", g=2, j=2)
                    nc.scalar.copy(tmp4, sps)
                    if r == 0:
                        nc.vector.tensor_mul(psb[:], tmp[:], mask4)
                    else:
                        nc.gpsimd.tensor_mul(psb[:], tmp[:], mask4)

                # intra lagged by 2 chunks so the mask never stalls the PE
                if c > 1:
                    intra(c - 2)

                rec[c] = {"psb": psb, "sp": sp,
                          "v": [v_sl(h) for h in range(HPC)]}
                rec.pop(c - 3, None)

            def intra(c):
                w, cl = c // CPW, c % CPW
                r = rec[c]
                sp = r["sp"]
                for h in range(HPC):
                    nc.tensor.matmul(
                        sp[:, HOFF[h]:HOFF[h] + 64],
                        r["psb"][:, 128 * h:128 * h + 128], r["v"][h],
                        start=False, stop=(h % 2 == 1),
                        skip_group_check=True,
                    )
                # out slots -> fp16 staging; DMA per half-window
                if ost_t[w] is None:
                    ost_t[w] = ostp.tile([128, WCOLS], f16,
                                         name=f"ost{w}", tag="ost")
                dst = ost_t[w].rearrange("p (c g j n) -> p c g j n",
                                         c=CPW, g=2, j=2)[:, cl]
                src = sp.rearrange("p (g q) -> p g q", g=2)[:, :, 0:384]
                src = src.rearrange("p g (j b) -> p g j b", j=2)[:, :, :, 0:64]
                if c % 2 == 0:
                    nc.scalar.copy(dst, src)
                else:
                    nc.vector.tensor_copy(dst, src)
                if cl % 4 == 3:
                    a = w * WCOLS + 256 * (cl - 3)
                    b = w * WCOLS + 256 * (cl + 1)
                    nc.sync.dma_start(
                        o[:, a:b], ost_t[w][:, 256 * (cl - 3):256 * (cl + 1)])
                    if cl == CPW - 1:
                        ost_t[w] = None

            # prologue: first chunks issued on separate queues so issue
            # overhead overlaps; compute starts after ~1 chunk of data
            def dma_qn_q(eng, w, clo, chi):
                a, b = 256 * clo, 256 * chi
                eng.dma_start(qnsb4[:, w, :, a:b], qn4[:, w, :, a:b])

            def dma_qt_q(eng, w, clo, chi):
                eng.dma_start(qtsb6[:, w, clo:chi, :, 64:192],
                              qt5[:, w, clo:chi, :, :])

            dma_qn_q(nc.sync, 0, 0, 1)
            dma_qt_q(nc.scalar, 0, 0, 1)
            dma_qn_q(nc.sync, 0, 1, 2)
            dma_qt_q(nc.scalar, 0, 1, 2)
            nc.scalar.dma_start(cst_sb[:], cst[:])
            dma_qn_q(nc.sync, 0, 2, 4)
            dma_qt_q(nc.scalar, 0, 2, 4)
            dma_qn_q(nc.sync, 0, 4, CPW)
            dma_qt_q(nc.scalar, 0, 4, CPW)

            for c in range(CH):
                w, cl = c // CPW, c % CPW
                if w < NW - 1:
                    if cl == 0:
                        dma_qt(w + 1, 0, CPW)
                    elif cl == 1:
                        dma_qn(w + 1, 0, CPW)
                body(c)
            intra(CH - 2)
            intra(CH - 1)

    nc.compile()
    return nc


_CACHE = {}


def _get_program():
    if "nc" not in _CACHE:
        _CACHE["nc"] = build_program()
    return _CACHE["nc"]


def _tables():
    n = np.arange(N, dtype=np.float64)
    tq = np.floor(n / 2.0) * 2.0
    freqs = 1.0 / (THETA ** (tq / N)) / (2.0 * math.pi)
    t = np.arange(T, dtype=np.float64)[:, None]
    ang = ((t * freqs[None, :]) % 1.0) * (2.0 * math.pi)
    scale = float(N) ** -0.25
    cc = (np.cos(ang) * scale).astype(np.float32)
    ss = (np.sin(ang) * scale).astype(np.float32)
    ss[:, 0::2] *= -1.0
    return cc, ss


def make_inputs(Q, V):
    """Full inputs -> list of per-core {'qn','qt','cst'} fp16 host arrays."""
    Q = np.asarray(Q, dtype=np.float32).reshape(NCORES, HPC, T, N)
    V = np.asarray(V, dtype=np.float32).reshape(NCORES, HPC, T, N)
    cc, ss = _tables()
    sq = np.empty_like(Q)
    sq[..., 0::2] = Q[..., 1::2]
    sq[..., 1::2] = Q[..., 0::2]
    qr = (Q * cc + sq * ss).astype(np.float16)  # scaled rope(Q)
    v16 = V.astype(np.float16)

    # natural: [core, h, w, cw, p, n] -> [core, p, w, (cw h n)]
    def nat(x):
        x = x.reshape(NCORES, HPC, NW, CPW, 128, N)
        return np.transpose(x, (0, 4, 2, 3, 1, 5))  # core p w cw h n

    qn_h = np.stack([nat(qr), nat(v16)], axis=3)  # core p w s cw h n
    qn_h = np.ascontiguousarray(qn_h.reshape(NCORES, 128, NW * 2 * WCOLS))

    # transposed: [core, n, w, (cw h t)]
    qt_h = qr.reshape(NCORES, HPC, NW, CPW, 128, N)
    qt_h = np.transpose(qt_h, (0, 5, 2, 3, 1, 4))  # core n w cw h t
    qt_h = np.ascontiguousarray(qt_h.reshape(NCORES, 64, NW * 2 * WCOLS))

    mu = np.triu(np.ones((128, 128), dtype=np.float16), k=1)
    cst = np.ascontiguousarray(np.concatenate([mu] * 4, axis=1))  # [128, 512]
    return [{"qn": qn_h[i], "qt": qt_h[i], "cst": cst}
            for i in range(NCORES)]


def unpack_out(results):
    """list of per-core {'o': [128, NW*WCOLS] fp16} -> [B,H,T,N] f32."""
    o = np.stack([r["o"] for r in results], axis=0)
    o = o.reshape(NCORES, 128, NW, CPW, HPC, N)
    o = np.transpose(o, (0, 4, 2, 3, 1, 5))  # [8, HPC, NW, CPW, 128, N]
    return np.ascontiguousarray(
        o.reshape(B, H, T, N).astype(np.float32))


def kernel(Q, V):
    from concourse.bass_utils import run_bass_kernel_spmd

    nc = _get_program()
    in_maps = make_inputs(Q, V)
    res = run_bass_kernel_spmd(nc, in_maps, core_ids=list(range(NCORES)))
    return unpack_out(res.results)
